# revision 17
# baseline (speedup 1.0000x reference)
"""MaskLinear kernel for 8x TRN2 NeuronCores.

Computes out[m,d] = sum_n weight[n] * masks[m,n] * x[n,d] + bias
 (= (masks * weight) @ x + bias), with x:[100000,256], masks:[64,100000].

Strategy: shard the contraction axis N across 8 cores. Each core gets a
12500-row slice (zero-padded to 12544 = 98*128 rows = "chunks" of 128),
computes a partial [2M,D] (two psum bank halves), and the host sums the
8 cores x 4 psum quarters and adds bias.

The host pre-folds weight into the transposed masks (wm = masks.T * w,
prescaled by 2^8 so fp16 stays in the normal range) so the device does
NO elementwise work: each chunk is a pair of f16 matmuls (lhsT=[128,64]
wm, rhs=[128,256] x) feeding the PE via concurrent column groups. The
packed DRAM layout gives one per-partition-contiguous DMA per group on
alternating queues; matmuls depend directly on the group DMA (no DVE
hop). Chunks 0..49 accumulate into psum bank A, drained (copy + output
DMA) mid-stream under the remaining input traffic; chunks 50..97 go to
bank B with a ramped-down tail ([...,4,2]) so only one chunk pair, one
copy and one small output DMA remain after the last input DMA lands.
"""

import numpy as np

import concourse.bacc as bacc
import concourse.mybir as mybir
from concourse import tile
from concourse.bass_utils import run_bass_kernel_spmd

N_CORES = 8
N = 100000
D = 256
M = 64
NS = N // N_CORES          # 12500 rows per shard
CHUNK = 128                # matmul contraction tile (partition dim)
C = -(-NS // CHUNK)        # 98 chunks
NP = C * CHUNK             # 12544 padded rows per shard
GW = M + D                 # packed row width: wm row + x row = 320
WSCALE = 256.0             # fp16 weight prescale (undone on host)

# Ramped DMA groups (in chunks). First four (50 chunks) accumulate into
# psum bank A, drained mid-stream; the rest (48) into bank B. The tail
# ramps down so the post-last-DMA chain is one pair + copy + out DMA.
# ENGS balances the two DMA rings (sync=50, scalar=48 chunks) so neither
# ring limps alone at the stream end, and both rings end with small
# groups (an unbalanced 58/40 split measurably ragged the stream tail).
GROUPS = [14, 14, 14, 8, 14, 14, 8, 6, 4, 2]
ENGS = ["s", "c", "s", "c", "c", "s", "c", "s", "c", "s"]
SPLIT = 4                  # groups[:SPLIT] -> bank A, rest -> bank B
NWARM = 6                  # HAM warmup matmuls (clock-gate ramp)

assert sum(GROUPS) == C
assert all(g % 2 == 0 for g in GROUPS)
A_CHUNKS = sum(GROUPS[:SPLIT])
assert A_CHUNKS % 2 == 0
assert sum(b for b, e in zip(GROUPS, ENGS) if e == "s") == 50

_STATE = {}


def _build_nc():
    nc = bacc.Bacc("TRN2", target_bir_lowering=False, debug=False,
                   num_devices=N_CORES)

    f16 = mybir.dt.float16
    f32 = mybir.dt.float32
    bf16 = mybir.dt.bfloat16

    pk = nc.dram_tensor("pk", [CHUNK, C * GW], f16, kind="ExternalInput")
    out = nc.dram_tensor("out", [2 * M, D], f32, kind="ExternalOutput")

    with tile.TileContext(nc) as tc:
        with (
            tc.tile_pool(name="cn", bufs=1) as cn,
            tc.tile_pool(name="gp", bufs=1) as gp,
            tc.tile_pool(name="pp", bufs=1, space="PSUM") as pp,
            tc.tile_pool(name="op", bufs=1) as op,
        ):
            if NWARM:
                # HAM warmup: junk bf16 matmuls keep the PE array busy while
                # the leading DMAs are in flight so the clock gate opens to
                # 8/8 before (or soon after) real work arrives.
                jz = cn.tile([CHUNK, 512], bf16)
                wz = cn.tile([CHUNK, 1], bf16)
                nc.vector.memset(jz[:], 0.0)
                nc.vector.memset(wz[:], 0.0)
                pwarm = pp.tile([1, 512], f32, tag="pwarm")
                for i in range(NWARM):
                    nc.tensor.matmul(pwarm[:], wz[:], jz[:],
                                     start=(i == 0), stop=(i == NWARM - 1))

            psA = pp.tile([2 * M, D], f32, tag="psA")
            psB = pp.tile([2 * M, D], f32, tag="psB")
            n_pairs_A = A_CHUNKS // 2
            n_pairs_B = (C - A_CHUNKS) // 2

            cbase = 0
            for g, B in enumerate(GROUPS):
                # wm goes into its OWN tile: if lhsT were a slice of the same
                # tile as rhs, LDWEIGHTS would contend with the running
                # matmul's rhs reads on the SBUF ports and the column-group
                # pair would serialize (426ns/pair instead of 213).
                wmt = gp.tile([CHUNK, B * M], f16, tag=f"wm{g}")
                xt = gp.tile([CHUNK, B * D], f16, tag=f"px{g}")
                eng = nc.sync if ENGS[g] == "s" else nc.scalar
                off = cbase * GW
                eng.dma_start(wmt[:], pk[:, off:off + B * M])
                eng.dma_start(xt[:], pk[:, off + B * M:off + B * GW])

                for b in range(0, B, 2):
                    c = cbase + b
                    if c < A_CHUNKS:
                        ps, cp, np_ = psA, c // 2, n_pairs_A
                    else:
                        ps, cp, np_ = psB, (c - A_CHUNKS) // 2, n_pairs_B
                    # Chunk pair: col groups 0-1 and 2-3 run concurrently,
                    # accumulating into disjoint psum partition halves.
                    nc.tensor.matmul(
                        ps[0:M, :],
                        wmt[:, b * M:(b + 1) * M],
                        xt[:, b * D:(b + 1) * D],
                        start=(cp == 0),
                        stop=(cp == np_ - 1),
                        tile_position=(0, 0),
                    )
                    nc.tensor.matmul(
                        ps[M:2 * M, :],
                        wmt[:, (b + 1) * M:(b + 2) * M],
                        xt[:, (b + 1) * D:(b + 2) * D],
                        start=(cp == 0),
                        stop=(cp == np_ - 1),
                        tile_position=(0, M),
                    )
                cbase += B
                if g == SPLIT:
                    # Bank A done: park it in SBUF (DVE is idle mid-stream;
                    # a DVE op can read at most ONE PSUM operand, so the
                    # final add needs this side in SBUF).
                    osbA = op.tile([2 * M, D], f32, tag="osbA")
                    nc.vector.tensor_copy(osbA[:], psA[:])
            # Final drain: bank A never round-trips to DRAM - the last DVE
            # op adds bank B (PSUM) to the parked bank A (same cost as a
            # copy: free size drives the cycles), then two parallel half
            # DMAs on sync+scalar move the single [128,256] result.
            osbB = op.tile([2 * M, D], f32, tag="osbB")
            nc.vector.tensor_add(osbB[:], psB[:], osbA[:])
            nc.sync.dma_start(out[0:M, :], osbB[0:M, :])
            nc.scalar.dma_start(out[M:2 * M, :], osbB[M:2 * M, :])
    nc.compile()
    return nc


def _get_nc():
    if "nc" not in _STATE:
        _STATE["nc"] = _build_nc()
    return _STATE["nc"]


def _shard_inputs(x, masks, weight):
    dt = np.dtype(np.float16)
    x = np.asarray(x, dtype=np.float32)
    masks = np.asarray(masks, dtype=np.float32)
    weight = np.asarray(weight, dtype=np.float32)
    # Fold the weight into the transposed masks on the host; fp16 prescale
    # by 2**8 keeps the tiny products (~1/sqrt(N)) in the normal range
    # (exact, undone after the gather).
    wmasks = masks.T * (weight * WSCALE)[:, None]   # [N, M] f32

    in_maps = []
    for s in range(N_CORES):
        lo = s * NS
        hi = lo + NS
        xs = np.zeros((NP, D), dt)
        xs[:NS] = x[lo:hi].astype(dt, copy=False)
        ms = np.zeros((NP, M), dt)
        ms[:NS] = wmasks[lo:hi].astype(dt, copy=False)
        # Pack per group: [128, B*M wm cols | B*D x cols], so each group
        # is one contiguous-per-partition DMA. Row (cbase*128 + p*B + b)
        # lands on partition p as sub-chunk b (same permutation for wm
        # and x, so the contraction is unaffected).
        blocks = []
        cbase = 0
        for B in GROUPS:
            r0, r1 = cbase * CHUNK, (cbase + B) * CHUNK
            blocks.append(ms[r0:r1].reshape(CHUNK, B * M))
            blocks.append(xs[r0:r1].reshape(CHUNK, B * D))
            cbase += B
        pkv = np.concatenate(blocks, axis=1)
        assert pkv.shape == (CHUNK, C * GW)
        in_maps.append({"pk": pkv})
    return in_maps


def _run(x, masks, weight, bias, **run_kwargs):
    in_maps = _shard_inputs(x, masks, weight)
    try:
        res = run_bass_kernel_spmd(
            _get_nc(), in_maps, core_ids=list(range(N_CORES)), **run_kwargs
        )
    except Exception:
        # The runtime occasionally reports a transient unrecoverable-device
        # error that clears on the next execution; retry once.
        res = run_bass_kernel_spmd(
            _get_nc(), in_maps, core_ids=list(range(N_CORES)), **run_kwargs
        )
    parts = np.stack([r["out"] for r in res.results])  # [8, 2M, 256]
    full = parts.sum(axis=0)
    full = full[:M] + full[M:2 * M]
    full = full * np.float32(1.0 / WSCALE)
    out = full + np.asarray(bias, dtype=np.float32)
    return out.astype(np.float32), res


def kernel(x, masks, weight, bias):
    out, _ = _run(x, masks, weight, bias)
    return out
